# revision 5
# baseline (speedup 1.0000x reference)
# Trainium2 Bass kernel for nn_MixtureOfExperts_37237366456694 — v3.
#
# Reference computation (B=4096, D=1024, H1=H2=4096, D_OUT=1024, K=8, G_H=512):
#   U[:,k,:] = MLP_k(x)                      (3-layer ReLU MLP per expert)
#   g        = softmax(gate_MLP(x))          (B, K)
#   Q        = cayley(A); B_k = Q[:, k*128:(k+1)*128]
#   V[:,k,:] = U[:,k,:] @ (B_k B_k^T)
#   out      = (sum_k g[:,k] * V[:,k,:]) @ Wo + bo
#
# Algebraic collapse (exact):
#   out[b] = sum_k g[b,k] * (h2_k[b] @ v_k + c_k) + bo
#   with v_k = W3_k @ (B_k B_k^T Wo), c_k = b3_k . (B_k B_k^T Wo).
# The third expert layer + projection + head fold into a matvec (host f64).
#
# Sharding: expert-parallel, one expert per core; gate batch-sharded (core k
# computes all-expert logits for its own 512 rows); host does the softmax
# combine, so there is no on-device collective.
#
# HW lessons baked in (all measured on-device):
#   * bf16 everywhere on the PE; N=512 streams sustain ~110ns/MM
#   * keep one PSUM bank per accumulation run (bank-pair interleaving
#     costs ~150ns/MM in HAM oscillation)
#   * HWDGE descriptor generation is ~68ns/descriptor per ring: weight
#     streams must use >=16KB descriptors (host-packed layouts) and spread
#     across rings (sync + gpsimd SWDGE)
#   * the folded v-matvec runs on the idle DVE (fused (h2*v)+acc), with 8
#     final fp32 ones-matvecs on the PE
import os

import numpy as np

P = 128


class _Cfg:
    def __init__(self, B=4096, D=1024, H=4096, GH=512, NT=512, SLAB=1024,
                 W2G=2, reps=1, ablate=None):
        self.B, self.D, self.H, self.GH, self.NT, self.SLAB = B, D, H, GH, NT, SLAB
        self.ablate = ablate  # None | noL1 | noL2 | resw2 (timing studies)
        self.DC = D // P      # d_in chunks
        self.HC = H // P      # hidden chunks (H1 == H2)
        self.GC = GH // P     # gate hidden chunks
        self.NSLAB = B // SLAB
        self.SN = SLAB // NT  # n-tiles per slab
        self.W2G = W2G        # fc tiles per packed W2 DMA
        self.reps = reps      # >1 only for differential benchmarking


def _np16():
    import ml_dtypes
    return ml_dtypes.bfloat16


def _build_nc(cfg):
    import concourse.bass as bass  # noqa: F401
    import concourse.mybir as mybir
    import concourse.tile as tile
    from concourse import bacc

    b16 = mybir.dt.bfloat16
    f32 = mybir.dt.float32
    Relu = mybir.ActivationFunctionType.Relu
    Alu = mybir.AluOpType

    B, DC, HC, GC, NT, SLAB, SN, NSLAB, W2G = (
        cfg.B, cfg.DC, cfg.HC, cfg.GC, cfg.NT, cfg.SLAB, cfg.SN, cfg.NSLAB,
        cfg.W2G)

    nc = bacc.Bacc(None, target_bir_lowering=False)
    # x, slab-packed: [sl, p, dc, b] (per-partition 16KB contiguous per slab)
    xsd = nc.dram_tensor("xs", (NSLAB, P, DC, SLAB), b16, kind="ExternalInput")
    # own 512-row gate block, packed: [p, dc, b]
    xgd = nc.dram_tensor("xg", (P, DC, NT), b16, kind="ExternalInput")
    # W1 packed in groups of W1G hc-tiles: [p, gw, j*DC*P + dc*P + m]
    W1G = 8
    W1d = nc.dram_tensor("W1", (P, HC // W1G, W1G * DC * P), b16,
                         kind="ExternalInput")
    # W2 packed in groups of W2G fc-tiles: [p, g, j*HC*P + hc*P + m]
    # (per-partition W2G*8KB contiguous per group)
    W2d = nc.dram_tensor("W2", (P, HC // W2G, W2G * HC * P), b16,
                         kind="ExternalInput")
    # f32 consts: [b1 (HC) | b2 (HC) | bg1 (GC) | ones (1)]
    NF = 2 * HC + GC + 1
    cfd = nc.dram_tensor("constf", (P, NF, 1), f32, kind="ExternalInput")
    # bf16 consts: [v (HC) | wg2 all-expert columns (GC*8)]
    NH = HC + GC * 8
    chd = nc.dram_tensor("consth", (P, NH, 1), b16, kind="ExternalInput")
    # [p, dc, gh] = Wg1[dc*P+p, gh]
    Wg1d = nc.dram_tensor("Wg1", (P, DC, cfg.GH), b16, kind="ExternalInput")
    out_s = nc.dram_tensor("out_s", (1, B), f32, kind="ExternalOutput")
    out_e = nc.dram_tensor("out_e", (8, NT), f32, kind="ExternalOutput")

    with tile.TileContext(nc) as tc:
        with (
            tc.tile_pool(name="const", bufs=1) as const,
            tc.tile_pool(name="xp", bufs=2) as xp,
            tc.tile_pool(name="w1p", bufs=2) as w1p,
            tc.tile_pool(name="w2p", bufs=2) as w2p,
            tc.tile_pool(name="h1p", bufs=1) as h1p,
            tc.tile_pool(name="h2p", bufs=3) as h2p,
            tc.tile_pool(name="accp", bufs=1) as accp,
            tc.tile_pool(name="outp", bufs=2) as outp,
            tc.tile_pool(name="mmps", bufs=6, space="PSUM") as mmps,
            tc.tile_pool(name="vps", bufs=2, space="PSUM") as vps,
        ):
            # --- constants ---
            wg1_t = const.tile((P, DC, cfg.GH), b16)
            nc.sync.dma_start(wg1_t[:], Wg1d[:])
            cf_t = const.tile((P, NF, 1), f32)
            nc.sync.dma_start(cf_t[:], cfd[:])
            ch_t = const.tile((P, NH, 1), b16)
            nc.sync.dma_start(ch_t[:], chd[:])
            xg_t = const.tile((P, DC, NT), b16)
            nc.sync.dma_start(xg_t[:], xgd[:])
            b1_t = cf_t[:, 0:HC, :]
            b2_t = cf_t[:, HC:2 * HC, :]
            bg1_t = cf_t[:, 2 * HC:2 * HC + GC, :]
            ones_t = cf_t[:, 2 * HC + GC:2 * HC + GC + 1, :]
            v_t = ch_t[:, 0:HC, :]

            # persistent gate-weighted accumulators, one per global n-tile
            acc = [accp.tile((P, NT), f32, name=f"acc{i}")
                   for i in range(NSLAB * SN)]

            h1_shared = None
            if cfg.ablate == "noL1":
                h1_shared = h1p.tile((P, HC, SLAB), b16, name="h1", tag="h1")
                nc.vector.memset(h1_shared[:], 0.0)

            for rep in range(cfg.reps):
                for a in acc:
                    nc.vector.memset(a[:], 0.0)

                # --- gate: own 512 rows, all 8 expert logits ---
                z1 = outp.tile((P, GC, NT), b16, name="z1", tag="z1")
                for gc in range(GC):
                    ps = mmps.tile((P, NT), f32, name="ps_g", tag="mm")
                    for dc in range(DC):
                        nc.tensor.matmul(
                            ps, wg1_t[:, dc, gc * P:(gc + 1) * P],
                            xg_t[:, dc, :],
                            start=(dc == 0), stop=(dc == DC - 1))
                    nc.scalar.activation(z1[:, gc, :], ps, Relu,
                                         bias=bg1_t[:, gc, :])
                lp8 = vps.tile((8, NT), f32, name="lp8", tag="vec")
                for gc in range(GC):
                    wg2_gc = ch_t[:, HC + gc * 8:HC + (gc + 1) * 8, 0]
                    nc.tensor.matmul(lp8, wg2_gc, z1[:, gc, :],
                                     start=(gc == 0), stop=(gc == GC - 1))
                lt = outp.tile((8, NT), f32, name="lt", tag="ot")
                nc.vector.tensor_copy(lt[:], lp8)
                nc.sync.dma_start(out_e[:, :], lt[:])

                for sl in range(NSLAB):
                    # --- x slab (packed: one 2MB DMA, 16KB/partition) ---
                    xt = xp.tile((P, DC, SLAB), b16, name="xt", tag="xt")
                    nc.sync.dma_start(xt[:], xsd[sl])

                    # --- layer 1: h1 = relu(x @ W1 + b1), transposed ---
                    if cfg.ablate == "noL1":
                        h1 = h1_shared
                    else:
                        h1 = h1p.tile((P, HC, SLAB), b16, name="h1", tag="h1")
                    l1_groups = ([] if cfg.ablate == "noL1"
                                 else list(range(HC // W1G)))
                    for gw in l1_groups:
                        w1s = w1p.tile((P, W1G * DC * P), b16, name="w1s",
                                       tag="w1s")
                        nc.scalar.dma_start(w1s[:], W1d[:, gw, :])
                        for j in range(W1G):
                            hc = gw * W1G + j
                            for n in range(SN):
                                ns = slice(n * NT, (n + 1) * NT)
                                ps = mmps.tile((P, NT), f32, name="ps1",
                                               tag="mm")
                                for dc in range(DC):
                                    w_off = (j * DC + dc) * P
                                    nc.tensor.matmul(ps,
                                                     w1s[:, w_off:w_off + P],
                                                     xt[:, dc, ns],
                                                     start=(dc == 0),
                                                     stop=(dc == DC - 1))
                                nc.scalar.activation(h1[:, hc, ns], ps, Relu,
                                                     bias=b1_t[:, hc, :])

                    # --- layer 2 + DVE-folded v-matvec ---
                    l2_groups = ([] if cfg.ablate == "noL2"
                                 else list(range(HC // W2G)))
                    w2res = None
                    for g in l2_groups:
                        if cfg.ablate == "resw2":
                            if w2res is None:
                                w2res = w2p.tile((P, W2G * HC * P), b16,
                                                 name="w2s", tag="w2s")
                                nc.sync.dma_start(w2res[:], W2d[:, 0, :])
                            w2s = w2res
                        else:
                            w2s = w2p.tile((P, W2G * HC * P), b16, name="w2s",
                                           tag="w2s")
                            eng = nc.sync if g % 2 == 0 else nc.gpsimd
                            eng.dma_start(w2s[:], W2d[:, g, :])
                        for j in range(W2G):
                            fc = g * W2G + j
                            for n in range(SN):
                                ns = slice(n * NT, (n + 1) * NT)
                                ps = mmps.tile((P, NT), f32, name="ps2",
                                               tag="mm")
                                for hc in range(HC):
                                    w_off = (j * HC + hc) * P
                                    nc.tensor.matmul(ps,
                                                     w2s[:, w_off:w_off + P],
                                                     h1[:, hc, ns],
                                                     start=(hc == 0),
                                                     stop=(hc == HC - 1))
                                h2t = h2p.tile((P, NT), b16, name="h2t",
                                               tag="h2t")
                                nc.scalar.activation(h2t[:], ps, Relu,
                                                     bias=b2_t[:, fc, :])
                                a = acc[sl * SN + n]
                                # acc += h2t * v[:, fc]   (fused on DVE)
                                nc.vector.scalar_tensor_tensor(
                                    a[:], h2t[:], v_t[:, fc, :], a[:],
                                    op0=Alu.mult, op1=Alu.add)

                # --- final partition-reduction: s = sum_p acc[p, :] ---
                for i in range(NSLAB * SN):
                    sp = vps.tile((1, NT), f32, name="sp", tag="vec")
                    nc.tensor.matmul(sp, ones_t[:, 0, :], acc[i][:],
                                     start=True, stop=True)
                    ot = outp.tile((1, NT), f32, name="ot", tag="ot")
                    nc.vector.tensor_copy(ot[:], sp)
                    nc.sync.dma_start(out_s[0:1, i * NT:(i + 1) * NT], ot[:])
    nc.compile()
    return nc


_STATE = {}
LAST_RESULTS = None
LAST_RUN_SECONDS = None


def _get_nc(cfg):
    key = (cfg.B, cfg.D, cfg.H, cfg.GH, cfg.NT, cfg.SLAB, cfg.W2G, cfg.reps,
           cfg.ablate)
    if key not in _STATE:
        _STATE[key] = _build_nc(cfg)
    return _STATE[key]


def _fold(W3, b3, A, Wo):
    """v_k = W3_k @ (B_k B_k^T Wo),  c_k = b3_k . (B_k B_k^T Wo) in float64."""
    A64 = A.astype(np.float64)
    S = A64 - A64.T
    I = np.eye(A.shape[0])
    Q = np.linalg.solve(I - S, I + S)
    K = W3.shape[0]
    sub = Q.shape[1] // K
    Bq = Q.reshape(Q.shape[0], K, sub)                      # [d, k, s]
    coef = np.einsum('dks,d->ks', Bq, Wo[:, 0].astype(np.float64))
    w = np.einsum('dks,ks->kd', Bq, coef)                   # (K, dim)
    v = np.einsum('kfd,kd->kf', W3.astype(np.float64), w)   # (K, H2)
    c = np.einsum('kd,kd->k', b3.astype(np.float64), w)     # (K,)
    return v, c


def _prep_in_maps(cfg, x, W1, b1, W2, b2, v, Wg1, bg1, Wg2, bg2):
    b16 = _np16()
    f32 = np.float32
    K = W1.shape[0]
    DC, HC, GC, NT, SLAB, NSLAB, W2G = (cfg.DC, cfg.HC, cfg.GC, cfg.NT,
                                        cfg.SLAB, cfg.NSLAB, cfg.W2G)

    # xT [p, dc, b]
    xT = x.astype(b16).T.reshape(DC, P, cfg.B).transpose(1, 0, 2)
    # slab-packed [sl, p, dc, b]
    xs = np.ascontiguousarray(
        xT.reshape(P, DC, NSLAB, SLAB).transpose(2, 0, 1, 3))
    # W1 packed groups [p, gw, j*DC*P + dc*P + m], W1G=8 hc-tiles per group
    W1G = 8
    W1p = np.ascontiguousarray(
        W1.astype(b16).reshape(K, DC, P, HC // W1G, W1G, P)
        .transpose(0, 2, 3, 4, 1, 5)   # k, p, gw, j, dc, m
        .reshape(K, P, HC // W1G, W1G * DC * P))
    # W2 fc-major [fc, p, hc, m] -> packed groups [p, g, j*HC*P...]
    W2f = W2.astype(b16).reshape(K, HC, P, HC, P).transpose(0, 3, 2, 1, 4)
    W2p = np.ascontiguousarray(
        W2f.reshape(K, HC // W2G, W2G, P, HC * P).transpose(0, 3, 1, 2, 4)
        .reshape(K, P, HC // W2G, W2G * HC * P))
    Wg1p = np.ascontiguousarray(
        Wg1.astype(b16).reshape(DC, P, cfg.GH).transpose(1, 0, 2))

    NF = 2 * HC + GC + 1
    constf = np.empty((K, P, NF, 1), f32)
    constf[:, :, 0:HC, 0] = b1.astype(f32).reshape(K, HC, P).transpose(0, 2, 1)
    constf[:, :, HC:2 * HC, 0] = (
        b2.astype(f32).reshape(K, HC, P).transpose(0, 2, 1))
    constf[:, :, 2 * HC:2 * HC + GC, 0] = bg1.astype(f32).reshape(GC, P).T[None]
    constf[:, :, 2 * HC + GC, 0] = 1.0
    NH = HC + GC * 8
    consth = np.empty((K, P, NH, 1), b16)
    consth[:, :, 0:HC, 0] = v.astype(b16).reshape(K, HC, P).transpose(0, 2, 1)
    wg2p = Wg2.astype(b16).reshape(GC, P, K).transpose(1, 0, 2).reshape(P, GC * K)
    consth[:, :, HC:, 0] = wg2p[None]

    in_maps = []
    for k in range(K):
        xg = np.ascontiguousarray(xT[:, :, k * NT:(k + 1) * NT])
        in_maps.append({
            "xs": xs,
            "xg": xg,
            "W1": W1p[k],
            "W2": W2p[k],
            "constf": constf[k],
            "consth": consth[k],
            "Wg1": Wg1p,
        })
    return in_maps


def kernel(x, W1, b1, W2, b2, W3, b3, Wg1, bg1, Wg2, bg2, A, Wo, bo):
    global LAST_RESULTS, LAST_RUN_SECONDS
    import time

    from concourse.bass_utils import run_bass_kernel_spmd

    cfg = _Cfg(B=x.shape[0], D=x.shape[1], H=W1.shape[2], GH=Wg1.shape[1])
    K = W1.shape[0]

    v, c = _fold(W3, b3, A, Wo)
    in_maps = _prep_in_maps(cfg, x, W1, b1, W2, b2, v, Wg1, bg1, Wg2, bg2)
    nc = _get_nc(cfg)

    t0 = time.time()
    res = run_bass_kernel_spmd(nc, in_maps, core_ids=list(range(K)))
    LAST_RUN_SECONDS = time.time() - t0
    LAST_RESULTS = res

    s = np.stack([r["out_s"][0] for r in res.results]).astype(np.float64)
    logit = np.concatenate([r["out_e"] for r in res.results],
                           axis=1).astype(np.float64)  # (8, B)
    e = np.exp(logit + bg2.astype(np.float64)[:, None])
    num = (e * (s + c[:, None])).sum(axis=0)
    den = e.sum(axis=0)
    out = num / den + float(bo[0])
    return out.astype(np.float32)[:, None]
